# revision 1
# baseline (speedup 1.0000x reference)
"""Trainium2 Bass kernel for nn_Attention_90744069030375.

Reference computation (per batch b, S=2048, D=1024):
    scores = (q @ k^T) * scale                      [S, S]
    attn_mask = max(pad_i, pad_j, causal_triu)      (pad = ~mask)
    scores -= 1e9 * attn_mask
    attn   = softmax(scores, -1)
    out    = attn @ k        (v = k)

Two numerical subtleties drive the design:

1. For a padded query row (mask[i]=False) every logit gets -1e9, which
   *mathematically* cancels in softmax -- but in fp32 ulp(1e9) = 64, so
   `scores - 1e9` collapses the row onto a 64-wide grid and softmax becomes
   uniform over the top bucket.  The grading reference runs in fp32 and has
   exactly this behavior, so we reproduce it: the additive bias is shipped
   as an exact fp32 {0, -1e9} tensor and applied to fp32 scores.

2. Bucket membership flips if our scores differ from the reference's by
   more than ~ulp-boundary distances, so QK^T must be near-fp32-accurate.
   A single fp16 pass (logit err ~1.6e-2) fails; a bf16 hi/lo 3-pass
   (q ~ qh+ql, k ~ kh+kl, scores = qh.kh + qh.kl + ql.kh, fp32 PSUM
   accumulation, logit err ~1e-4) gives aggregate rel err ~2e-4.

Sharding: data-parallel over batch -- 8 batches -> 8 NeuronCores, one
batch each, no collectives.  Host pre-marshals per core: transposed bf16
hi/lo q and k ([D,S], lhsT/rhs for QK^T), fp16 k ([S,D], rhs for attn@K),
and the fp32 mask bias ([S,S], streamed per row-tile).  Softmax runs on
ACT (exp with fused row-sum via accum_out) + DVE (max/reciprocal/bias
add).  attn is transposed for the PV matmul with the DMA xbar transpose
(SBUF->SBUF fp16); PV accumulates over 16 key blocks into fp32 PSUM and
rows are scaled by 1/rowsum on the way out.
"""

import numpy as np
import ml_dtypes

import concourse.bass as bass
import concourse.bacc as bacc
import concourse.mybir as mybir
from concourse.bass_utils import run_bass_kernel_spmd
from concourse.tile import TileContext

B, S, D = 8, 2048, 1024
P = 128                 # partitions / M-tile rows
NQ = S // P             # 16 query row-tiles
ND = D // P             # 8 contraction tiles for QK^T
NJ = S // 512           # 4 key column tiles of 512
BF16 = mybir.dt.bfloat16
F16 = mybir.dt.float16
F32 = mybir.dt.float32


def build_bass(reps=1, qk_order="d", sc_bufs=6, pv_bufs=2):
    nc = bacc.Bacc()
    qTh = nc.dram_tensor("qTh", [D, S], BF16, kind="ExternalInput")
    qTl = nc.dram_tensor("qTl", [D, S], BF16, kind="ExternalInput")
    kTh = nc.dram_tensor("kTh", [D, S], BF16, kind="ExternalInput")
    kTl = nc.dram_tensor("kTl", [D, S], BF16, kind="ExternalInput")
    kpv = nc.dram_tensor("kpv", [S, D], F16, kind="ExternalInput")
    masku8 = nc.dram_tensor("masku8", [S, S], mybir.dt.uint8,
                            kind="ExternalInput")
    out = nc.dram_tensor("out", [S, D], F32, kind="ExternalOutput")

    with TileContext(nc) as tc:
        with (
            tc.tile_pool(name="weights", bufs=1) as wpool,
            tc.tile_pool(name="work", bufs=2) as work,
            tc.tile_pool(name="stats", bufs=3) as stats,
            tc.tile_pool(name="scores", bufs=sc_bufs, space="PSUM") as scores_pool,
            tc.tile_pool(name="pv", bufs=pv_bufs, space="PSUM") as pv_pool,
        ):
            # ---- persistent operands (merged tiles: one slot per group) --
            # [:, d*S:(d+1)*S] of qTh_all is the [128, S] d-th contraction
            # tile of q-hi, etc.  Loads are issued on the SP HWDGE queues in
            # the order the matmul loop consumes them (k column-chunks in n
            # order first) so the first banks can start after ~4MB instead
            # of ~20MB.  The xbar attn transposes live on the ACT HWDGE
            # queues instead (xbar-mode transitions serialize per queue).
            qTh_all = wpool.tile([P, ND * S], BF16, tag="qTh")
            qTl_all = wpool.tile([P, ND * S], BF16, tag="qTl")
            kTh_all = wpool.tile([P, ND * S], BF16, tag="kTh")
            kTl_all = wpool.tile([P, ND * S], BF16, tag="kTl")
            kpv_all = wpool.tile([P, NQ * D], F16, tag="kpv")
            for n in range(NJ):
                nsl = slice(n * 512, (n + 1) * 512)
                for d in range(ND):
                    sl = slice(d * P, (d + 1) * P)
                    nc.sync.dma_start(
                        out=kTh_all[:, d * S:(d + 1) * S][:, nsl],
                        in_=kTh[sl, nsl])
                    nc.sync.dma_start(
                        out=kTl_all[:, d * S:(d + 1) * S][:, nsl],
                        in_=kTl[sl, nsl])
                    if n == 0:
                        # q slices for the first few row-tiles, interleaved
                        # with kT n=0 so the d-th matmul of the first bank
                        # can start as soon as its own d-chunks land
                        nc.sync.dma_start(
                            out=qTh_all[:, d * S:(d + 1) * S][:, 0:512],
                            in_=qTh[sl, 0:512])
                        nc.sync.dma_start(
                            out=qTl_all[:, d * S:(d + 1) * S][:, 0:512],
                            in_=qTl[sl, 0:512])
            for j in range(NQ):
                nc.sync.dma_start(out=kpv_all[:, j * D:(j + 1) * D],
                                  in_=kpv[j * P:(j + 1) * P, :])
            for d in range(ND):
                sl = slice(d * P, (d + 1) * P)
                nc.sync.dma_start(
                    out=qTh_all[:, d * S:(d + 1) * S][:, 512:S],
                    in_=qTh[sl, 512:S])
                nc.sync.dma_start(
                    out=qTl_all[:, d * S:(d + 1) * S][:, 512:S],
                    in_=qTl[sl, 512:S])

            # ---- main loop over query row-tiles --------------------------
            pending_pv = []
            # reps>1 repeats the whole computation back-to-back in one NEFF
            # (benchmarking only: marginal time per rep = steady-state time)
            for m_rep in range(reps * NQ):
                m = m_rep % NQ
                # mask rows stream as u8; expand to the exact fp32 {0,-1e9}
                # additive bias on DVE (16MB -> 2MB of DMA)
                bias_u8 = work.tile([P, S], mybir.dt.uint8, tag="bias_u8")
                nc.gpsimd.dma_start(
                    out=bias_u8, in_=masku8[m * P:(m + 1) * P, :])
                bias = work.tile([P, S], F32, tag="bias")
                nc.vector.tensor_scalar_mul(bias, bias_u8, float(-1e9))

                # QK^T: 3 bf16 passes accumulate in fp32 PSUM; n-outer so
                # each bank finishes early and softmax pipelines behind the
                # next bank's matmuls.
                msl = slice(m * P, (m + 1) * P)
                nsls = [slice(n * 512, (n + 1) * 512) for n in range(NJ)]
                sc = [scores_pool.tile([P, 512], F32, name=f"sc{n}", tag="sc")
                      for n in range(NJ)]
                pmax = stats.tile([P, NJ], F32, tag="pmax")
                # qk_order: how many PSUM banks share one LDWEIGHTS of the
                # q slice -- "n": 1 bank (64 LDW/m, per-bank completion),
                # "pair": 2 banks (32 LDW/m), "d": all 4 banks (16 LDW/m,
                # banks complete together).
                group = {"n": 1, "pair": 2, "d": NJ}[qk_order]
                for g0 in range(0, NJ, group):
                    ns = range(g0, min(g0 + group, NJ))
                    for d in range(ND):
                        qh_d = qTh_all[:, d * S:(d + 1) * S][:, msl]
                        for n in ns:
                            nc.tensor.matmul(t := sc[n], qh_d,
                                             kTh_all[:, d * S:(d + 1) * S][:, nsls[n]],
                                             start=(d == 0), stop=False)
                            nc.tensor.matmul(t, qh_d,
                                             kTl_all[:, d * S:(d + 1) * S][:, nsls[n]],
                                             start=False, stop=False)
                    for d in range(ND):
                        ql_d = qTl_all[:, d * S:(d + 1) * S][:, msl]
                        for n in ns:
                            nc.tensor.matmul(sc[n], ql_d,
                                             kTh_all[:, d * S:(d + 1) * S][:, nsls[n]],
                                             start=False, stop=(d == ND - 1))
                    for n in ns:
                        # exact fp32 reference bias (in-place on PSUM, DVE)
                        nc.vector.tensor_add(sc[n], sc[n], bias[:, nsls[n]])
                        nc.vector.reduce_max(
                            pmax[:, n:n + 1], sc[n], axis=mybir.AxisListType.X)

                negmax = stats.tile([P, 1], F32, tag="negmax")
                nc.vector.reduce_max(
                    negmax, pmax, axis=mybir.AxisListType.X, negate=True)

                # exp(x - rowmax) on ACT, row-sums fused via accum_out
                attn = work.tile([P, S], F16, tag="attn")
                psums = stats.tile([P, NJ], F32, tag="psums")
                for n in range(NJ):
                    nc.scalar.activation(
                        out=attn[:, n * 512:(n + 1) * 512],
                        in_=sc[n],
                        func=mybir.ActivationFunctionType.Exp,
                        bias=negmax,
                        scale=1.0,
                        accum_out=psums[:, n:n + 1],
                    )
                recip = stats.tile([P, 1], F32, tag="recip")
                nc.vector.reduce_sum(recip, psums, axis=mybir.AxisListType.X)
                nc.vector.reciprocal(recip, recip)

                # transpose attn for PV (DMA xbar): attnT[:, jb, :] is the
                # [j=128, i=128] lhsT block for key block jb
                attnT = work.tile([P, NQ, P], F16, tag="attnT", bufs=3)
                for n in range(NJ):
                    nc.scalar.dma_start(
                        out=attnT[:, 4 * n:4 * (n + 1), :],
                        in_=attn[:, n * 512:(n + 1) * 512],
                        transpose=True,
                    )

                # PV is emitted AFTER the next tile's QK^T (deferred
                # closure): both PV(m) and QK(m+1) gate on softmax(m), and
                # with PV(m) at lower scheduler priority it stays available
                # to fill the softmax latency of the FINAL tile, which
                # otherwise leaves the PE idle ~10us at the kernel tail.
                def make_pv(m, attnT, recip):
                    def emit_pv():
                        pv = [pv_pool.tile([P, 512], F32, name=f"pv{nn}",
                                           tag="pv") for nn in range(2)]
                        for jb in range(NQ):
                            lhsT = attnT[:, jb, :]
                            for nn in range(2):
                                nc.tensor.matmul(
                                    pv[nn],
                                    lhsT,
                                    kpv_all[:, jb * D:(jb + 1) * D][
                                        :, nn * 512:(nn + 1) * 512],
                                    start=(jb == 0),
                                    stop=(jb == NQ - 1),
                                )
                        # normalize rows and store
                        osb = work.tile([P, D], F32, name="osb", tag="osb", bufs=1)
                        for nn in range(2):
                            nc.vector.tensor_scalar_mul(
                                osb[:, nn * 512:(nn + 1) * 512], pv[nn],
                                recip)
                        nc.sync.dma_start(
                            out=out[m * P:(m + 1) * P, :], in_=osb)
                    return emit_pv

                if len(pending_pv) == 2:
                    pending_pv.pop(0)()
                pending_pv.append(make_pv(m, attnT, recip))
            for f in pending_pv:
                f()

    return nc


_NC_CACHE = None


def _get_nc():
    global _NC_CACHE
    if _NC_CACHE is None:
        _NC_CACHE = build_bass()
        if not _NC_CACHE.is_finalized():
            _NC_CACHE.finalize()
    return _NC_CACHE


def make_in_maps(q, k, mask, scale):
    bf = ml_dtypes.bfloat16
    triu = np.triu(np.ones((S, S), np.float32), k=1)
    in_maps = []
    s = float(np.asarray(scale))
    for b in range(B):
        qs = (q[b] * s).astype(np.float32)
        qh = qs.astype(bf)
        ql = (qs - qh.astype(np.float32)).astype(bf)
        kh = k[b].astype(bf)
        kl = (k[b] - kh.astype(np.float32)).astype(bf)
        pad = (~mask[b]).astype(np.float32)
        am = np.maximum(np.maximum(pad[:, None], pad[None, :]), triu)
        in_maps.append({
            "qTh": np.ascontiguousarray(qh.T),
            "qTl": np.ascontiguousarray(ql.T),
            "kTh": np.ascontiguousarray(kh.T),
            "kTl": np.ascontiguousarray(kl.T),
            "kpv": np.ascontiguousarray(k[b].astype(np.float16)),
            "masku8": am.astype(np.uint8),
        })
    return in_maps


def kernel(q, k, mask, scale, _want_trace=False, **trace_kwargs):
    nc = _get_nc()
    in_maps = make_in_maps(
        np.asarray(q), np.asarray(k), np.asarray(mask), np.asarray(scale))
    res = run_bass_kernel_spmd(
        nc, in_maps, list(range(B)), trace=_want_trace, **trace_kwargs)
    outs = np.stack([res.results[i]["out"] for i in range(B)], axis=0)
    outs = outs.astype(np.float32)
    if _want_trace:
        return outs, res
    return outs



# revision 4
# speedup vs baseline: 2.7751x; 2.7751x over previous
"""Trainium2 Bass kernel for nn_Attention_90744069030375.

Row/column-permuted masked causal attention; per-core steady state is
PE-bound at ~162us (cost model) / ~200us (HW marginal), 2.7x over the
previous 542us version.

Reference (per batch, S=2048, D=1024):
    scores = (q @ k^T) * scale; bias = -1e9 * max(pad_i, pad_j, triu)
    attn = softmax(scores + bias); out = attn @ k     (v = k)

Key numerics: for a padded query row (mask[i]=False) every logit gets
-1e9; in fp32 ulp(1e9)=64 so `scores - 1e9` collapses the row onto a
64-wide grid and softmax becomes uniform over the top bucket.  Bucket
membership needs |score error| << distance-to-bucket-boundary, hence
high-precision QK^T for padded rows.  Valid rows are ordinary masked
softmax (masked entries underflow to weight exactly 0) and tolerate
1-pass fp16.

Design: per batch, permute rows AND columns valid-first (softmax is
row-wise, so row permutation commutes; keys/values permuted together so
the valid x valid block becomes standard causal attention in permuted
space).  Tiles of 128 query rows:
  - t < VT  (pure valid):  1-pass fp16 QK over cols <= 128(t+1) only
    (causal compaction -- exact: skipped cols have weight exactly 0),
    triangular -1e9 bias on the diagonal block, flash softmax, PV over
    (t+1) key blocks.
  - VT <= t < MIXHI (mixed valid/padded): 2-pass fp16 QK (q hi/lo x k),
    exact per-element bias streamed as u8 (covers causal+pad for valid
    rows, all -1e9 for padded rows), exp path, full PV.
  - t >= MIXHI (pure padded): 2-pass QK, scalar -1e9 add on PSUM
    (the fp32 collapse), exp path, full PV.  2-pass error (dropped
    q*(k-fp16(k)) term, ~4.5e-3) flips ~a bucket membership per few
    hundred rows; measured end-to-end rel err 6.8e-3 vs the 2e-2 gate.

The second ("lo") QK pass runs in fp8e5m2 with DoubleRow perf mode
(2 k-tiles per matmul); e5m2's exponent range fits the ~2^-12-scale q
residuals without rescaling, so it accumulates into the same PSUM
group.  DMA is split across the SP HWDGE queues (kT/qT/out), the Pool
SWDGE channel (kpv, fp8 operands, bias), and the ACT queues (only the
attn xbar transposes -- ACT-engine time is needed for exp).

VT = floor(min_b V_b/128), MIXHI = ceil(max_b V_b/128) are data-driven
compile-time constants (program cached per (VT, MIXHI)).  Sharding:
data-parallel, one batch per NeuronCore, no collectives.
"""

import numpy as np
import ml_dtypes

import concourse.bass as bass
import concourse.bacc as bacc
import concourse.mybir as mybir
from concourse.bass_utils import run_bass_kernel_spmd
from concourse.tile import TileContext

B, S, D = 8, 2048, 1024
P = 128                 # partitions / tile rows
NQ = S // P             # 16 query row-tiles
ND = D // P             # 8 contraction chunks
NJ = S // 512           # 4 key column banks of 512
F16 = mybir.dt.float16
F32 = mybir.dt.float32
F8E5 = mybir.dt.float8e5


def build_bass(VT, MIXHI, reps=1, sc_bufs=6, pv_bufs=2, lo8=True):
    """VT: # pure-valid causal tiles; MIXHI: first pure-padded tile.

    lo8: run the QK lo pass (q residual x k) in fp8e5m2 with DoubleRow
    (2 k-tiles per matmul, ~1.4x) instead of fp16.  e5m2's exponent range
    reaches 2^-16, so the ~2^-12-scale q residuals need no rescaling and
    the pass accumulates straight into the same PSUM group.
    """
    VB = VT * P                      # valid-block columns
    SG = S - VB                      # rows handled by general (mixed+padded) tiles
    MIXN = MIXHI - VT                # number of mixed tiles
    LODT = F8E5 if lo8 else F16

    nc = bacc.Bacc()
    qT = nc.dram_tensor("qT", [D, S], F16, kind="ExternalInput")
    qTl = nc.dram_tensor("qTl", [D, SG], LODT, kind="ExternalInput") if SG else None
    kT8 = nc.dram_tensor("kT8", [D, S], F8E5, kind="ExternalInput") if lo8 else None
    kT = nc.dram_tensor("kT", [D, S], F16, kind="ExternalInput")
    kpv = nc.dram_tensor("kpv", [S, D], F16, kind="ExternalInput")
    tribias = nc.dram_tensor("tribias", [P, P], F32, kind="ExternalInput")
    biasmix = (nc.dram_tensor("biasmix", [MIXN * P, S], mybir.dt.uint8,
                              kind="ExternalInput") if MIXN else None)
    out = nc.dram_tensor("out", [S, D], F32, kind="ExternalOutput")

    # processing order: V0 first (needs almost no data), then alternate
    # padded (heavy) and valid (light) tiles; mixed tiles last.
    valid_ts = list(range(VT))
    padded_ts = list(range(MIXHI, NQ))
    mixed_ts = list(range(VT, MIXHI))
    order = []
    vi_, pi_ = 0, 0
    if valid_ts:
        order.append(("V", valid_ts[0])); vi_ = 1
    while vi_ < len(valid_ts) or pi_ < len(padded_ts):
        if pi_ < len(padded_ts):
            order.append(("P", padded_ts[pi_])); pi_ += 1
        if vi_ < len(valid_ts):
            order.append(("V", valid_ts[vi_])); vi_ += 1
    order += [("M", t) for t in mixed_ts]

    with TileContext(nc) as tc:
        with (
            tc.tile_pool(name="weights", bufs=1) as wpool,
            tc.tile_pool(name="work", bufs=2) as work,
            tc.tile_pool(name="stats", bufs=3) as stats,
            tc.tile_pool(name="scores", bufs=sc_bufs, space="PSUM") as scores_pool,
            tc.tile_pool(name="pv", bufs=pv_bufs, space="PSUM") as pv_pool,
        ):
            # ---- persistent operands (merged tiles, one slot per group) --
            qT_all = wpool.tile([P, ND * S], F16, tag="qT")
            qTl_all = (wpool.tile([P, ND, SG], LODT, name="qTl_all", tag="qTl")
                       if SG else None)
            kT8_all = (wpool.tile([P, ND, S], F8E5, name="kT8_all", tag="kT8")
                       if lo8 else None)
            kT_all = wpool.tile([P, ND * S], F16, tag="kT")
            kpv_all = wpool.tile([P, NQ * D], F16, tag="kpv")
            trib = wpool.tile([P, P], F32, tag="trib")

            def qslice(d, lo, hi):      # qT_all view, global row range
                return qT_all[:, d * S + lo: d * S + hi]

            def qlslice(d, lo, hi):     # qTl_all view, global rows >= VB
                return qTl_all[:, d, (lo - VB):(hi - VB)]

            def kslice(d, lo, hi):
                return kT_all[:, d * S + lo: d * S + hi]

            # DMA issue order ~ consumption order, split across the two
            # HWDGE engines so the ~22MB of QK operands stream on parallel
            # channels: SP carries kT n0/n1 + qT; ACT carries kT n2/n3 +
            # the (later-needed) fp8 lo-pass operands; kpv + bias ride the
            # gpsimd SWDGE channel.  Output stores go on SP (post-prefix).
            nc.sync.dma_start(out=trib, in_=tribias[:, :])
            for j in range(NQ):
                nc.gpsimd.dma_start(out=kpv_all[:, j * D:(j + 1) * D],
                                    in_=kpv[j * P:(j + 1) * P, :])
            first_rows = [(t * P, (t + 1) * P) for _, t in order[:3]]
            done = set(first_rows)
            rest = []
            for _, t in order[3:]:
                lo, hi = t * P, (t + 1) * P
                if (lo, hi) in done:
                    continue
                done.add((lo, hi))
                rest.append((lo, hi))

            def load_q(eng, lo, hi):
                for d in range(ND):
                    dsl = slice(d * P, (d + 1) * P)
                    eng.dma_start(out=qslice(d, lo, hi), in_=qT[dsl, lo:hi])

            def load_ql(eng, lo, hi):
                if lo >= VB and SG:
                    for d in range(ND):
                        dsl = slice(d * P, (d + 1) * P)
                        eng.dma_start(out=qlslice(d, lo, hi),
                                      in_=qTl[dsl, lo - VB:hi - VB])

            # SP: kT n-major interleaved with q rows in consumption
            # order.  (ACT stays free for exp + xbar transposes -- its
            # HWDGE dispatch competes with activation compute.)
            lo0, hi0 = first_rows[0]
            for d in range(ND):
                dsl = slice(d * P, (d + 1) * P)
                nc.sync.dma_start(out=qslice(d, lo0, hi0), in_=qT[dsl, lo0:hi0])
                nc.sync.dma_start(out=kslice(d, 0, 512), in_=kT[dsl, 0:512])
            for (lo, hi) in first_rows[1:]:
                load_q(nc.sync, lo, hi)
            for n in range(1, NJ):
                nsl = slice(n * 512, (n + 1) * 512)
                for d in range(ND):
                    dsl = slice(d * P, (d + 1) * P)
                    nc.sync.dma_start(out=kslice(d, n * 512, (n + 1) * 512),
                                      in_=kT[dsl, nsl])
                for (lo, hi) in rest[(n - 1) * 2:n * 2]:
                    load_q(nc.sync, lo, hi)
            for (lo, hi) in rest[(NJ - 1) * 2:]:
                load_q(nc.sync, lo, hi)

            # Pool (SWDGE): fp8 lo-pass operands -- needed only by each
            # tile's second pass, latency-tolerant.
            if lo8:
                for n in range(NJ):
                    nsl = slice(n * 512, (n + 1) * 512)
                    for d in range(ND):
                        dsl = slice(d * P, (d + 1) * P)
                        nc.gpsimd.dma_start(out=kT8_all[:, d, nsl],
                                            in_=kT8[dsl, nsl])
                    if n == 0:
                        for (lo, hi) in first_rows:
                            load_ql(nc.gpsimd, lo, hi)
            else:
                for (lo, hi) in first_rows:
                    load_ql(nc.gpsimd, lo, hi)
            for (lo, hi) in rest:
                load_ql(nc.gpsimd, lo, hi)

            # ---- per-tile emission ---------------------------------------
            pending_pv = []

            def emit_tile(kind, t, group=None):
                lo, hi = t * P, (t + 1) * P
                if kind == "V":
                    CB = (t + 1) * P            # causal column extent
                    NB = (CB + 511) // 512      # banks in use
                else:
                    CB, NB = S, NJ
                bw = [min(CB, (n + 1) * 512) - n * 512 for n in range(NB)]

                if kind == "M":
                    bias_u8 = work.tile([P, S], mybir.dt.uint8, tag="bias_u8")
                    nc.gpsimd.dma_start(
                        out=bias_u8, in_=biasmix[lo - VB:hi - VB, :])
                    bias = work.tile([P, S], F32, tag="bias")
                    nc.vector.tensor_scalar_mul(bias, bias_u8, float(-1e9))

                # QK^T: fp16, hi pass (+ lo pass for general tiles).
                # `group` = how many PSUM banks share one pass over the q
                # d-chunks; smaller groups start before all kT banks land.
                G = group or NB
                sc = [scores_pool.tile([P, 512], F32, name=f"sc{n}", tag="sc")
                      for n in range(NB)]
                for g0 in range(0, NB, G):
                    ns = range(g0, min(g0 + G, NB))
                    for d in range(ND):
                        qh_d = qslice(d, lo, hi)
                        for n in ns:
                            nc.tensor.matmul(
                                sc[n][:, :bw[n]], qh_d,
                                kslice(d, n * 512, n * 512 + bw[n]),
                                start=(d == 0),
                                stop=(d == ND - 1 and kind == "V"))
                if kind != "V":
                    if lo8:
                        for j in range(ND // 2):
                            l8 = qTl_all[:, 2 * j:2 * j + 2, (lo - VB):(hi - VB)]
                            for n in range(NB):
                                nc.tensor.matmul(
                                    sc[n][:, :bw[n]], l8,
                                    kT8_all[:, 2 * j:2 * j + 2,
                                            n * 512:n * 512 + bw[n]],
                                    start=False, stop=(j == ND // 2 - 1),
                                    perf_mode=mybir.MatmulPerfMode.DoubleRow)
                    else:
                        for d in range(ND):
                            ql_d = qlslice(d, lo, hi)
                            for n in range(NB):
                                nc.tensor.matmul(
                                    sc[n][:, :bw[n]], ql_d,
                                    kslice(d, n * 512, n * 512 + bw[n]),
                                    start=False, stop=(d == ND - 1))

                # bias / collapse, then per-bank row-max
                pmax = stats.tile([P, NB], F32, tag="pmax")
                for n in range(NB):
                    if kind == "V":
                        dlo = t * P - n * 512   # diag block offset in bank n
                        if 0 <= dlo < bw[n]:
                            nc.vector.tensor_add(
                                sc[n][:, dlo:dlo + P],
                                sc[n][:, dlo:dlo + P], trib)
                    elif kind == "M":
                        nc.vector.tensor_add(
                            sc[n], sc[n], bias[:, n * 512:(n + 1) * 512])
                    else:   # pure padded: exact fp32 collapse onto 64-grid
                        nc.vector.tensor_scalar_add(sc[n], sc[n], float(-1e9))
                    nc.vector.reduce_max(
                        pmax[:, n:n + 1], sc[n][:, :bw[n]],
                        axis=mybir.AxisListType.X)

                negmax = stats.tile([P, 1], F32, tag="negmax")
                nc.vector.reduce_max(
                    negmax, pmax, axis=mybir.AxisListType.X, negate=True)

                # exp(x - rowmax) on ACT, fused row-sums
                attn = work.tile([P, S], F16, tag="attn")
                psums = stats.tile([P, NJ], F32, tag="psums")
                for n in range(NB):
                    nc.scalar.activation(
                        out=attn[:, n * 512:n * 512 + bw[n]],
                        in_=sc[n][:, :bw[n]],
                        func=mybir.ActivationFunctionType.Exp,
                        bias=negmax, scale=1.0,
                        accum_out=psums[:, n:n + 1])
                recip = stats.tile([P, 1], F32, tag="recip")
                nc.vector.reduce_sum(
                    recip, psums[:, :NB], axis=mybir.AxisListType.X)
                nc.vector.reciprocal(recip, recip)

                # transpose attn for PV (DMA xbar, SBUF->SBUF fp16)
                attnT = work.tile([P, NQ, P], F16, tag="attnT", bufs=3)
                for n in range(NB):
                    nc.scalar.dma_start(
                        out=attnT[:, 4 * n:4 * n + bw[n] // P, :],
                        in_=attn[:, n * 512:n * 512 + bw[n]],
                        transpose=True)

                NKB = (t + 1) if kind == "V" else NQ   # PV key blocks

                def make_pv(lo=lo, attnT=attnT, recip=recip, NKB=NKB):
                    def emit_pv():
                        pv = [pv_pool.tile([P, 512], F32, name=f"pv{nn}",
                                           tag="pv") for nn in range(2)]
                        for jb in range(NKB):
                            lhsT = attnT[:, jb, :]
                            for nn in range(2):
                                nc.tensor.matmul(
                                    pv[nn], lhsT,
                                    kpv_all[:, jb * D + nn * 512:
                                            jb * D + (nn + 1) * 512],
                                    start=(jb == 0), stop=(jb == NKB - 1))
                        osb = work.tile([P, D], F32, name="osb", tag="osb",
                                        bufs=2)
                        for nn in range(2):
                            nc.vector.tensor_scalar_mul(
                                osb[:, nn * 512:(nn + 1) * 512], pv[nn], recip)
                        nc.sync.dma_start(out=out[lo:lo + P, :], in_=osb)
                    return emit_pv

                if len(pending_pv) == 2:
                    pending_pv.pop(0)()
                pending_pv.append(make_pv())

            for r in range(reps):
                for oi, (kind, t) in enumerate(order):
                    # first two heavy tiles: pair-grouped QK so bank 0/1
                    # matmuls start before kT banks 2/3 finish streaming
                    g = 2 if (r == 0 and kind != "V" and oi < 4) else None
                    emit_tile(kind, t, group=g)
            for f in pending_pv:
                f()

    return nc


_NC_CACHE = {}


def _get_nc(VT, MIXHI, reps=1):
    key = (VT, MIXHI, reps)
    if key not in _NC_CACHE:
        nc = build_bass(VT, MIXHI, reps=reps)
        if not nc.is_finalized():
            nc.finalize()
        _NC_CACHE[key] = nc
    return _NC_CACHE[key]


def plan_split(mask):
    Vs = mask.sum(1)
    VT = int(min(Vs) // P)
    MIXHI = int(-(-int(max(Vs)) // P))
    MIXHI = max(MIXHI, VT + 1) if MIXHI < NQ else MIXHI
    return VT, MIXHI


def make_in_maps(q, k, mask, scale, VT, MIXHI, lo8=True):
    f16 = ml_dtypes.float16 if hasattr(ml_dtypes, "float16") else np.float16
    e5 = ml_dtypes.float8_e5m2
    lodt = e5 if lo8 else f16
    VB, MIXN = VT * P, MIXHI - VT
    s = float(np.asarray(scale))
    tri = (np.triu(np.ones((P, P), np.float32), k=1) * np.float32(-1e9)
           ).astype(np.float32)
    in_maps, perms = [], []
    for b in range(B):
        vi = np.where(mask[b])[0]
        pi = np.where(~mask[b])[0]
        perm = np.concatenate([vi, pi])
        V = len(vi)
        perms.append(perm)
        qp = (q[b] * s).astype(np.float32)[perm]
        kp = k[b][perm].astype(np.float32)
        qh = qp.astype(f16)
        ql = (qp - qh.astype(np.float32)).astype(lodt)
        # mixed-tile bias: valid row r allows cols c <= r; padded rows none
        if MIXN:
            rows = np.arange(VB, MIXHI * P)
            cols = np.arange(S)
            allowed = (cols[None, :] <= rows[:, None]) & (rows[:, None] < V)
            bm = (~allowed).astype(np.uint8)
        in_map = {
            "qT": np.ascontiguousarray(qh.T),
            "kT": np.ascontiguousarray(kp.astype(f16).T),
            "kpv": np.ascontiguousarray(kp.astype(f16)),
            "tribias": tri,
        }
        if VB < S:
            in_map["qTl"] = np.ascontiguousarray(ql[VB:].T)
        if lo8:
            in_map["kT8"] = np.ascontiguousarray(kp.astype(e5).T)
        if MIXN:
            in_map["biasmix"] = bm
        in_maps.append(in_map)
    return in_maps, perms


def kernel(q, k, mask, scale, _want_trace=False, _reps=1, **trace_kwargs):
    q, k, mask = np.asarray(q), np.asarray(k), np.asarray(mask)
    VT, MIXHI = plan_split(mask)
    nc = _get_nc(VT, MIXHI, reps=_reps)
    in_maps, perms = make_in_maps(q, k, mask, np.asarray(scale), VT, MIXHI)
    res = run_bass_kernel_spmd(
        nc, in_maps, list(range(B)), trace=_want_trace, **trace_kwargs)
    outs = np.empty((B, S, D), np.float32)
    for b in range(B):
        outs[b][perms[b]] = res.results[b]["out"].astype(np.float32)
    if _want_trace:
        return outs, res
    return outs
